# revision 3
# baseline (speedup 1.0000x reference)
"""Trainium2 Bass kernel for nn_Criterion_29386166239267.

The reference loss ends with ``return loss[-1]``: the scalar output depends
ONLY on the last batch row (row 4095) of each (4096, 2048) float32 input.
So instead of data-parallel processing of all 4096 rows, the kernel ships
just that row (7 x 2048 floats = 56 KB) to one NeuronCore, computes the four
order-invariant row reductions on device, and combines them into the scalar
loss on the host:

    S1 = sum exp(z_post)               z_post  = mu + sqrt(sigma) * eps_post
    S2 = sum exp(z_prior)              z_prior = prior_mu + sqrt(prior_sigma) * eps_prior
    T1 = sum exp(z_prior) * (z_prior - z_post)
    L1 = sum log(sigma)
    L2 = sum (target_y - mu)^2 / sigma

    kl   = (T1/S2 + log(S1) - log(S2)) / nt
    loss = 0.5 * (LOG_2PI + (L1 + L2)/nt) + kl

(The max-subtraction in log_softmax is skipped: |z| < ~8 for any plausible
inputs of this problem, so exp() is well within f32 range.)

Each 2048-long row is laid out as [128 partitions x 16] in SBUF; free-axis
sums ride for free on the ACT/DVE ops (accum_out), and the final 128-way
partition sum of the five accumulator columns is one PE matmul with a ones
vector.
"""

import numpy as np

BS = 4096
NT = 2048
P = 128
F = NT // P  # 16
LOG_2PI = float(np.log(2.0 * np.pi))

_NAMES = (
    "prior_mu",
    "prior_sigma",
    "mu",
    "sigma",
    "target_y",
    "eps_post",
    "eps_prior",
)

_PROG = None


def _build_program():
    import concourse.bacc as bacc
    import concourse.mybir as mybir
    import concourse.tile as tile

    dt = mybir.dt.float32
    Act = mybir.ActivationFunctionType
    Alu = mybir.AluOpType

    nc = bacc.Bacc(
        "TRN2", target_bir_lowering=False, debug=False, enable_asserts=False
    )
    x_dram = nc.dram_tensor("x", [P, 7 * F], dt, kind="ExternalInput")
    out_dram = nc.dram_tensor("partials", [1, 5], dt, kind="ExternalOutput")

    with tile.TileContext(nc) as tc:
        with (
            tc.tile_pool(name="pool", bufs=1) as pool,
            tc.tile_pool(name="psum", bufs=1, space="PSUM") as psum,
        ):
            x = pool.tile([P, 7 * F], dt)
            nc.sync.dma_start(x[:], x_dram[:])
            pm, psig, mu, sg, ty, ep, epr = (
                x[:, i * F : (i + 1) * F] for i in range(7)
            )

            cols = pool.tile([P, 5], dt)
            ones = pool.tile([P, 1], dt)
            nc.vector.memset(ones[:], 1.0)

            # z_post = mu + sqrt(sigma) * eps_post; z_prior likewise
            sq_s = pool.tile([P, F], dt)
            nc.scalar.sqrt(sq_s[:], sg)
            sq_p = pool.tile([P, F], dt)
            nc.scalar.sqrt(sq_p[:], psig)
            zpo = pool.tile([P, F], dt)
            nc.vector.tensor_mul(zpo[:], sq_s[:], ep)
            nc.vector.tensor_add(zpo[:], zpo[:], mu)
            zpr = pool.tile([P, F], dt)
            nc.vector.tensor_mul(zpr[:], sq_p[:], epr)
            nc.vector.tensor_add(zpr[:], zpr[:], pm)

            # S1, S2 via ACT exp with free-axis accumulate
            ea = pool.tile([P, F], dt)
            nc.scalar.activation(ea[:], zpo[:], Act.Exp, accum_out=cols[:, 0:1])
            eb = pool.tile([P, F], dt)
            nc.scalar.activation(eb[:], zpr[:], Act.Exp, accum_out=cols[:, 1:2])

            # T1 = sum exp(z_prior) * (z_prior - z_post)
            # (tensor_tensor_reduce and vector.reciprocal are custom DVE ops
            # that crash this runtime — use plain mul + reduce_sum, and
            # 1/sigma = exp(-log(sigma)) on ACT instead.)
            dd = pool.tile([P, F], dt)
            nc.vector.tensor_sub(dd[:], zpr[:], zpo[:])
            t1 = pool.tile([P, F], dt)
            nc.vector.tensor_mul(t1[:], eb[:], dd[:])
            nc.vector.reduce_sum(cols[:, 2:3], t1[:], axis=mybir.AxisListType.X)

            # L1 = sum log(sigma)
            lg = pool.tile([P, F], dt)
            nc.scalar.activation(lg[:], sg, Act.Ln, accum_out=cols[:, 3:4])

            # L2 = sum (target_y - mu)^2 / sigma
            inv = pool.tile([P, F], dt)
            nc.scalar.activation(inv[:], lg[:], Act.Exp, scale=-1.0)
            r = pool.tile([P, F], dt)
            nc.vector.tensor_sub(r[:], ty, mu)
            r2 = pool.tile([P, F], dt)
            nc.scalar.square(r2[:], r[:])
            l2 = pool.tile([P, F], dt)
            nc.vector.tensor_mul(l2[:], r2[:], inv[:])
            nc.vector.reduce_sum(cols[:, 4:5], l2[:], axis=mybir.AxisListType.X)

            # 128-way partition sum of the 5 accumulator columns in one matmul
            acc = psum.tile([1, 5], dt)
            nc.tensor.matmul(acc[:], ones[:], cols[:])
            res = pool.tile([1, 5], dt)
            nc.vector.tensor_copy(res[:], acc[:])
            nc.sync.dma_start(out_dram[:], res[:])

    nc.compile()
    return nc


def _pack_last_rows(inputs) -> np.ndarray:
    x = np.empty((P, 7 * F), dtype=np.float32)
    for i, name in enumerate(_NAMES):
        row = np.asarray(inputs[name])[-1]
        x[:, i * F : (i + 1) * F] = np.asarray(row, dtype=np.float32).reshape(P, F)
    return x


def run_partials(x: np.ndarray, **kwargs):
    """Run the device program on the packed [128, 112] input; returns
    (partials[5] float64, BassKernelResults)."""
    global _PROG
    if _PROG is None:
        _PROG = _build_program()
    from concourse.bass_utils import run_bass_kernel_spmd

    res = run_bass_kernel_spmd(_PROG, [{"x": x}], [0], **kwargs)
    partials = np.asarray(res.results[0]["partials"], dtype=np.float64).ravel()
    return partials, res


def _combine(partials: np.ndarray) -> np.ndarray:
    s1, s2, t1, l1, l2 = partials
    kl = (t1 / s2 + np.log(s1) - np.log(s2)) / NT
    loss = 0.5 * (LOG_2PI + (l1 + l2) / NT) + kl
    return np.asarray(loss, dtype=np.float32)


def kernel(**inputs) -> np.ndarray:
    partials, _ = run_partials(_pack_last_rows(inputs))
    return _combine(partials)


# revision 5
# speedup vs baseline: 1.1154x; 1.1154x over previous
"""Trainium2 Bass kernel for nn_Criterion_29386166239267.

The reference loss ends with ``return loss[-1]``: the scalar output depends
ONLY on the last batch row (row 4095) of each (4096, 2048) float32 input.
So instead of data-parallel processing of all 4096 rows, the kernel ships
just that row (7 x 2048 floats = 56 KB) to one NeuronCore, computes the four
order-invariant row reductions on device, and combines them into the scalar
loss on the host:

    S1 = sum exp(z_post)               z_post  = mu + sqrt(sigma) * eps_post
    S2 = sum exp(z_prior)              z_prior = prior_mu + sqrt(prior_sigma) * eps_prior
    T1 = sum exp(z_prior) * (z_prior - z_post)
    L1 = sum log(sigma)
    L2 = sum (target_y - mu)^2 / sigma

    kl   = (T1/S2 + log(S1) - log(S2)) / nt
    loss = 0.5 * (LOG_2PI + (L1 + L2)/nt) + kl

(The max-subtraction in log_softmax is skipped: |z| < ~8 for any plausible
inputs of this problem, so exp() is well within f32 range.)

Each 2048-long row is laid out as [128 partitions x 16] in SBUF; free-axis
sums ride for free on the ACT/DVE ops (accum_out), and the final 128-way
partition sum of the five accumulator columns is one PE matmul with a ones
vector.
"""

import numpy as np

BS = 4096
NT = 2048
P = 128
F = NT // P  # 16
LOG_2PI = float(np.log(2.0 * np.pi))

_NAMES = (
    "prior_mu",
    "prior_sigma",
    "mu",
    "sigma",
    "target_y",
    "eps_post",
    "eps_prior",
)

_PROG = None


def _build_program():
    import concourse.bacc as bacc
    import concourse.mybir as mybir
    import concourse.tile as tile

    dt = mybir.dt.float32
    Act = mybir.ActivationFunctionType
    Alu = mybir.AluOpType

    nc = bacc.Bacc(
        "TRN2", target_bir_lowering=False, debug=False, enable_asserts=False
    )
    x_dram = nc.dram_tensor("x", [P, 7 * F], dt, kind="ExternalInput")
    out_dram = nc.dram_tensor("partials", [P, 5], dt, kind="ExternalOutput")

    with tile.TileContext(nc) as tc:
        with tc.tile_pool(name="pool", bufs=1) as pool:
            x = pool.tile([P, 7 * F], dt)
            nc.sync.dma_start(x[:], x_dram[:])
            pm, psig, mu, sg, ty, ep, epr = (
                x[:, i * F : (i + 1) * F] for i in range(7)
            )

            # Trigger the EXP and LN activation-table loads (~1.3us each,
            # serial on ACT) while the input DMA is still in flight, by
            # issuing two tiny activations on a const tile that's ready
            # immediately.  Only these two tables are used: sqrt and 1/x
            # are computed as exp(+-0.5/-1 * ln(x)).
            warm_src = nc.const_aps.scalar_like(1.0, x[:1, 0:1])
            warm = pool.tile([1, 1], dt)
            nc.scalar.activation(warm[:], warm_src, Act.Exp)
            nc.scalar.activation(warm[:], warm_src, Act.Ln)

            # W holds the five summand tiles; one grouped reduce at the end.
            # cols: 0=exp(z_post) 1=exp(z_prior) 2=exp(z_prior)*(z_prior-z_post)
            #       3=log(sigma)  4=(ty-mu)^2/sigma
            W = pool.tile([P, 5 * F], dt)

            lgs = W[:, 3 * F : 4 * F]
            nc.scalar.activation(lgs, sg, Act.Ln)
            lgp = pool.tile([P, F], dt)
            nc.scalar.activation(lgp[:], psig, Act.Ln)
            sq_s = pool.tile([P, F], dt)
            nc.scalar.activation(sq_s[:], lgs, Act.Exp, scale=0.5)
            sq_p = pool.tile([P, F], dt)
            nc.scalar.activation(sq_p[:], lgp[:], Act.Exp, scale=0.5)
            inv = pool.tile([P, F], dt)
            nc.scalar.activation(inv[:], lgs, Act.Exp, scale=-1.0)

            # z_post = mu + sqrt(sigma) * eps_post; z_prior likewise
            zpo = pool.tile([P, F], dt)
            nc.vector.tensor_mul(zpo[:], sq_s[:], ep)
            nc.vector.tensor_add(zpo[:], zpo[:], mu)
            zpr = pool.tile([P, F], dt)
            nc.vector.tensor_mul(zpr[:], sq_p[:], epr)
            nc.vector.tensor_add(zpr[:], zpr[:], pm)

            nc.scalar.activation(W[:, 0:F], zpo[:], Act.Exp)
            nc.scalar.activation(W[:, F : 2 * F], zpr[:], Act.Exp)

            dd = pool.tile([P, F], dt)
            nc.vector.tensor_sub(dd[:], zpr[:], zpo[:])
            nc.vector.tensor_mul(W[:, 2 * F : 3 * F], W[:, F : 2 * F], dd[:])

            r = pool.tile([P, F], dt)
            nc.vector.tensor_sub(r[:], ty, mu)
            r2 = pool.tile([P, F], dt)
            nc.vector.tensor_mul(r2[:], r[:], r[:])
            nc.vector.tensor_mul(W[:, 4 * F : 5 * F], r2[:], inv[:])

            # one grouped free-axis reduce: [P, 5, F] -> [P, 5]
            res = pool.tile([P, 5], dt)
            nc.vector.reduce_sum(
                res[:],
                W[:].rearrange("p (q f) -> p q f", q=5),
                axis=mybir.AxisListType.X,
            )
            nc.sync.dma_start(out_dram[:], res[:])

    nc.compile()
    return nc


def _pack_last_rows(inputs) -> np.ndarray:
    x = np.empty((P, 7 * F), dtype=np.float32)
    for i, name in enumerate(_NAMES):
        row = np.asarray(inputs[name])[-1]
        x[:, i * F : (i + 1) * F] = np.asarray(row, dtype=np.float32).reshape(P, F)
    return x


def run_partials(x: np.ndarray, **kwargs):
    """Run the device program on the packed [128, 112] input; returns
    (partials[5] float64, BassKernelResults)."""
    global _PROG
    if _PROG is None:
        _PROG = _build_program()
    from concourse.bass_utils import run_bass_kernel_spmd

    res = run_bass_kernel_spmd(_PROG, [{"x": x}], [0], **kwargs)
    # [128, 5] per-partition partial sums -> 5 row sums
    partials = np.asarray(res.results[0]["partials"], dtype=np.float64).sum(axis=0)
    return partials, res


def _combine(partials: np.ndarray) -> np.ndarray:
    s1, s2, t1, l1, l2 = partials
    kl = (t1 / s2 + np.log(s1) - np.log(s2)) / NT
    loss = 0.5 * (LOG_2PI + (l1 + l2) / NT) + kl
    return np.asarray(loss, dtype=np.float32)


def kernel(**inputs) -> np.ndarray:
    partials, _ = run_partials(_pack_last_rows(inputs))
    return _combine(partials)


# revision 7
# speedup vs baseline: 1.2121x; 1.0867x over previous
"""Trainium2 Bass kernel for nn_Criterion_29386166239267.

The reference loss ends with ``return loss[-1]``: the scalar output depends
ONLY on the last batch row (row 4095) of each (4096, 2048) float32 input.
So instead of data-parallel processing of all 4096 rows, the kernel ships
just that row (7 x 2048 floats = 56 KB) to one NeuronCore, computes the four
order-invariant row reductions on device, and combines them into the scalar
loss on the host:

    S1 = sum exp(z_post)               z_post  = mu + sqrt(sigma) * eps_post
    S2 = sum exp(z_prior)              z_prior = prior_mu + sqrt(prior_sigma) * eps_prior
    T1 = sum exp(z_prior) * (z_prior - z_post)
    L1 = sum log(sigma)
    L2 = sum (target_y - mu)^2 / sigma

    kl   = (T1/S2 + log(S1) - log(S2)) / nt
    loss = 0.5 * (LOG_2PI + (L1 + L2)/nt) + kl

(The max-subtraction in log_softmax is skipped: |z| < ~8 for any plausible
inputs of this problem, so exp() is well within f32 range.)

Each 2048-long row is laid out as [128 partitions x 16] in SBUF; free-axis
sums ride for free on the ACT/DVE ops (accum_out), and the final 128-way
partition sum of the five accumulator columns is one PE matmul with a ones
vector.
"""

import numpy as np

BS = 4096
NT = 2048
P = 128
F = NT // P  # 16
LOG_2PI = float(np.log(2.0 * np.pi))

_NAMES = (
    "prior_mu",
    "prior_sigma",
    "mu",
    "sigma",
    "target_y",
    "eps_post",
    "eps_prior",
)

_PROG = None


def _build_program_raw():
    """Raw Bass (no TileContext): hand-rolled semaphores, minimal epilogue.

    Engine plan — Sync: DMA in/out; Scalar (ACT): 2x Ln then 5x Exp (one
    LN->EXP table transition; LN table load hidden under the input DMA by a
    warm-up Ln on a const tile); Vector (DVE): elementwise + final grouped
    reduce.  Per-engine sems count completions in order, so one wait >= k
    covers all earlier ops of that engine (in-order completion).
    """
    import concourse.bass as bass
    import concourse.mybir as mybir

    dt = mybir.dt.float32
    Act = mybir.ActivationFunctionType

    nc = bass.Bass(
        "TRN2", target_bir_lowering=False, debug=False, enable_asserts=False
    )
    x_dram = nc.dram_tensor("x", [P, 7 * F], dt, kind="ExternalInput")
    out_dram = nc.dram_tensor("partials", [P, 5], dt, kind="ExternalOutput")

    def sb(name, shape):
        return nc.alloc_sbuf_tensor(name, shape, dt).ap()

    x = sb("xt", [P, 7 * F])
    pm, psig, mu, sg, ty, ep, epr = (x[:, i * F : (i + 1) * F] for i in range(7))
    W = sb("W", [P, 5 * F])  # 0=exp(zpo) 1=exp(zpr) 2=eb*(zpr-zpo) 3=ln(sg) 4=r2/sg
    lgs = W[:, 3 * F : 4 * F]
    lgp = sb("lgp", [P, F])
    sq_s = sb("sq_s", [P, F])
    sq_p = sb("sq_p", [P, F])
    inv = sb("inv", [P, F])
    zpo = sb("zpo", [P, F])
    zpr = sb("zpr", [P, F])
    dd = sb("dd", [P, F])
    r = sb("r", [P, F])
    r2 = sb("r2", [P, F])
    res = sb("res", [P, 5])
    warm = sb("warm", [1, 1])
    warm_src = nc.const_aps.tensor(1.0, (1, 1))

    with (
        nc.Block() as block,
        nc.semaphore("dsem") as dsem,
        nc.semaphore("ssem") as ssem,
        nc.semaphore("vsem") as vsem,
    ):

        @block.sync
        def _(sync):
            sync.dma_start(out=x, in_=x_dram[:]).then_inc(dsem, 16)
            sync.wait_ge(vsem, 10)
            sync.dma_start(out=out_dram[:], in_=res).then_inc(dsem, 16)
            sync.wait_ge(dsem, 32)

        @block.scalar
        def _(scalar):
            # s1: triggers the LN table load during the input DMA
            scalar.activation(warm, warm_src, Act.Ln).then_inc(ssem, 1)
            scalar.wait_ge(dsem, 16)
            scalar.activation(lgs, sg, Act.Ln).then_inc(ssem, 1)  # s2
            scalar.activation(lgp, psig, Act.Ln).then_inc(ssem, 1)  # s3
            scalar.wait_ge(ssem, 3)  # own-pipeline RAW: lgs/lgp complete
            # EXP table load slots in here, right after the Ln group
            scalar.activation(sq_s, lgs, Act.Exp, scale=0.5).then_inc(ssem, 1)  # s4
            scalar.activation(sq_p, lgp, Act.Exp, scale=0.5).then_inc(ssem, 1)  # s5
            scalar.activation(inv, lgs, Act.Exp, scale=-1.0).then_inc(ssem, 1)  # s6
            scalar.wait_ge(vsem, 4)
            scalar.activation(W[:, 0:F], zpo, Act.Exp).then_inc(ssem, 1)  # s7
            scalar.wait_ge(vsem, 6)
            scalar.activation(W[:, F : 2 * F], zpr, Act.Exp).then_inc(ssem, 1)  # s8

        @block.vector
        def _(vector):
            vector.wait_ge(dsem, 16)
            vector.tensor_sub(r, ty, mu).then_inc(vsem, 1)  # v1
            vector.wait_ge(vsem, 1)
            vector.tensor_mul(r2, r, r).then_inc(vsem, 1)  # v2
            vector.wait_ge(ssem, 4)
            vector.tensor_mul(zpo, sq_s, ep).then_inc(vsem, 1)  # v3
            vector.wait_ge(vsem, 3)
            vector.tensor_add(zpo, zpo, mu).then_inc(vsem, 1)  # v4
            vector.wait_ge(ssem, 5)
            vector.tensor_mul(zpr, sq_p, epr).then_inc(vsem, 1)  # v5
            vector.wait_ge(vsem, 5)
            vector.tensor_add(zpr, zpr, pm).then_inc(vsem, 1)  # v6
            vector.wait_ge(vsem, 6)
            vector.tensor_sub(dd, zpr, zpo).then_inc(vsem, 1)  # v7
            vector.wait_ge(ssem, 6)
            vector.tensor_mul(W[:, 4 * F : 5 * F], r2, inv).then_inc(vsem, 1)  # v8
            vector.wait_ge(ssem, 8)
            vector.wait_ge(vsem, 7)
            vector.tensor_mul(W[:, 2 * F : 3 * F], W[:, F : 2 * F], dd).then_inc(
                vsem, 1
            )  # v9
            vector.wait_ge(vsem, 9)
            vector.reduce_sum(
                res,
                W.rearrange("p (q f) -> p q f", q=5),
                axis=mybir.AxisListType.X,
            ).then_inc(vsem, 1)  # v10

    return nc


def _build_program():
    import concourse.bacc as bacc
    import concourse.mybir as mybir
    import concourse.tile as tile

    dt = mybir.dt.float32
    Act = mybir.ActivationFunctionType
    Alu = mybir.AluOpType

    nc = bacc.Bacc(
        "TRN2", target_bir_lowering=False, debug=False, enable_asserts=False
    )
    x_dram = nc.dram_tensor("x", [P, 7 * F], dt, kind="ExternalInput")
    out_dram = nc.dram_tensor("partials", [P, 5], dt, kind="ExternalOutput")

    with tile.TileContext(nc) as tc:
        with tc.tile_pool(name="pool", bufs=1) as pool:
            x = pool.tile([P, 7 * F], dt)
            nc.sync.dma_start(x[:], x_dram[:])
            pm, psig, mu, sg, ty, ep, epr = (
                x[:, i * F : (i + 1) * F] for i in range(7)
            )

            # Trigger the EXP and LN activation-table loads (~1.3us each,
            # serial on ACT) while the input DMA is still in flight, by
            # issuing two tiny activations on a const tile that's ready
            # immediately.  Only these two tables are used: sqrt and 1/x
            # are computed as exp(+-0.5/-1 * ln(x)).
            warm_src = nc.const_aps.scalar_like(1.0, x[:1, 0:1])
            warm = pool.tile([1, 1], dt)
            nc.scalar.activation(warm[:], warm_src, Act.Exp)
            nc.scalar.activation(warm[:], warm_src, Act.Ln)

            # W holds the five summand tiles; one grouped reduce at the end.
            # cols: 0=exp(z_post) 1=exp(z_prior) 2=exp(z_prior)*(z_prior-z_post)
            #       3=log(sigma)  4=(ty-mu)^2/sigma
            W = pool.tile([P, 5 * F], dt)

            lgs = W[:, 3 * F : 4 * F]
            nc.scalar.activation(lgs, sg, Act.Ln)
            lgp = pool.tile([P, F], dt)
            nc.scalar.activation(lgp[:], psig, Act.Ln)
            sq_s = pool.tile([P, F], dt)
            nc.scalar.activation(sq_s[:], lgs, Act.Exp, scale=0.5)
            sq_p = pool.tile([P, F], dt)
            nc.scalar.activation(sq_p[:], lgp[:], Act.Exp, scale=0.5)
            inv = pool.tile([P, F], dt)
            nc.scalar.activation(inv[:], lgs, Act.Exp, scale=-1.0)

            # z_post = mu + sqrt(sigma) * eps_post; z_prior likewise
            zpo = pool.tile([P, F], dt)
            nc.vector.tensor_mul(zpo[:], sq_s[:], ep)
            nc.vector.tensor_add(zpo[:], zpo[:], mu)
            zpr = pool.tile([P, F], dt)
            nc.vector.tensor_mul(zpr[:], sq_p[:], epr)
            nc.vector.tensor_add(zpr[:], zpr[:], pm)

            nc.scalar.activation(W[:, 0:F], zpo[:], Act.Exp)
            nc.scalar.activation(W[:, F : 2 * F], zpr[:], Act.Exp)

            dd = pool.tile([P, F], dt)
            nc.vector.tensor_sub(dd[:], zpr[:], zpo[:])
            nc.vector.tensor_mul(W[:, 2 * F : 3 * F], W[:, F : 2 * F], dd[:])

            r = pool.tile([P, F], dt)
            nc.vector.tensor_sub(r[:], ty, mu)
            r2 = pool.tile([P, F], dt)
            nc.vector.tensor_mul(r2[:], r[:], r[:])
            nc.vector.tensor_mul(W[:, 4 * F : 5 * F], r2[:], inv[:])

            # one grouped free-axis reduce: [P, 5, F] -> [P, 5]
            res = pool.tile([P, 5], dt)
            nc.vector.reduce_sum(
                res[:],
                W[:].rearrange("p (q f) -> p q f", q=5),
                axis=mybir.AxisListType.X,
            )
            nc.sync.dma_start(out_dram[:], res[:])

    nc.compile()
    return nc


def _pack_last_rows(inputs) -> np.ndarray:
    x = np.empty((P, 7 * F), dtype=np.float32)
    for i, name in enumerate(_NAMES):
        row = np.asarray(inputs[name])[-1]
        x[:, i * F : (i + 1) * F] = np.asarray(row, dtype=np.float32).reshape(P, F)
    return x


def run_partials(x: np.ndarray, **kwargs):
    """Run the device program on the packed [128, 112] input; returns
    (partials[5] float64, BassKernelResults)."""
    global _PROG
    if _PROG is None:
        _PROG = _build_program_raw()
    from concourse.bass_utils import run_bass_kernel_spmd

    res = run_bass_kernel_spmd(_PROG, [{"x": x}], [0], **kwargs)
    # [128, 5] per-partition partial sums -> 5 row sums
    partials = np.asarray(res.results[0]["partials"], dtype=np.float64).sum(axis=0)
    return partials, res


def _combine(partials: np.ndarray) -> np.ndarray:
    s1, s2, t1, l1, l2 = partials
    kl = (t1 / s2 + np.log(s1) - np.log(s2)) / NT
    loss = 0.5 * (LOG_2PI + (l1 + l2) / NT) + kl
    return np.asarray(loss, dtype=np.float32)


def kernel(**inputs) -> np.ndarray:
    partials, _ = run_partials(_pack_last_rows(inputs))
    return _combine(partials)
